# revision 4
# baseline (speedup 1.0000x reference)
"""Chamfer loss Trainium2 Bass kernel.

Problem: gts [8, 4096, 256], preds [8, 4096, 256] (f32) ->
    loss = sum_b [ sum_m min_n P_b[n,m] + sum_n min_m P_b[n,m] ]
    where P_b[n,m] = ||gts[b,n] - preds[b,m]||^2.

Strategy (one batch element per NeuronCore, 8 cores):
  P[n,m] = xx[n] + yy[m] - 2*Z[n,m],  Z = x @ y^T.
  On device we compute V = Z - 0.5*yy[m] - 0.5*xx[n] = -P/2 directly:
    * Z via fp32r matmuls (x^T, y^T fed D-major; contraction over D in
      two K=128 chunks),
    * the -0.5*yy[m] term via one extra K=1 matmul row (ones (x) -0.5*yy),
    * the -0.5*xx[n] term via the ScalarE per-partition bias during the
      PSUM -> fp16 SBUF copy.
  Then min-reductions of P become max-reductions of V:
    rowmax[n] = max_m V  (free-dim max tree on VectorE)
    colmax[m] = max_n V  (elementwise running max across n-chunks, then a
                          PE transpose + free-dim reduce at the end)
  loss_b = -2 * (sum rowmax + sum colmax); partials summed on host.
"""

import numpy as np
from contextlib import ExitStack

import concourse.bass as bass
import concourse.mybir as mybir
import concourse.tile as tile
from concourse import bacc
from concourse.masks import make_identity
from concourse.bass_utils import run_bass_kernel_spmd

N_CORES = 8
B, N, M, D = 8, 4096, 4096, 256
MBLK = 512  # matmul moving-operand block (one PSUM bank of fp32)


def build_chamfer(n=N, m=M, d=D, n_dma_groups=4):
    """Build the per-core Bass module. x: n rows, y: m rows, dim d."""
    assert d == 256, "kernel hardcodes two K=128 contraction chunks"
    assert n % 128 == 0 and m % MBLK == 0 and m % 128 == 0
    nchunk = n // 128
    nmb = m // MBLK
    f32, f32r, f16 = mybir.dt.float32, mybir.dt.float32r, mybir.dt.float16

    nc = bacc.Bacc("TRN2", target_bir_lowering=False, debug=False)
    xt = nc.dram_tensor("xt", [2, 128, n], f32r, kind="ExternalInput").ap()
    yt = nc.dram_tensor("yt", [2, 128, m], f32r, kind="ExternalInput").ap()
    xxb = nc.dram_tensor("xxb", [128, nchunk], f32, kind="ExternalInput").ap()
    yya = nc.dram_tensor("yya", [1, m], f16, kind="ExternalInput").ap()
    out = nc.dram_tensor("out", [1, 1], f32, kind="ExternalOutput").ap()

    with tile.TileContext(nc) as tc, ExitStack() as ctx:
        singles = ctx.enter_context(tc.tile_pool(name="singles", bufs=1))
        vh_pool = ctx.enter_context(tc.tile_pool(name="vh", bufs=2))
        tree_pool = ctx.enter_context(tc.tile_pool(name="tree", bufs=2))
        ps_pool = ctx.enter_context(tc.tile_pool(name="ps", bufs=4, space="PSUM"))
        tps_pool = ctx.enter_context(tc.tile_pool(name="tps", bufs=2, space="PSUM"))
        fps_pool = ctx.enter_context(tc.tile_pool(name="fps", bufs=1, space="PSUM"))

        # Static SBUF residents
        xs0 = singles.tile([128, n], f32r, name="xs0")
        xs1 = singles.tile([128, n], f32r, name="xs1")
        ys0 = singles.tile([128, m], f32r, name="ys0")
        ys1 = singles.tile([128, m], f32r, name="ys1")
        yya_s = singles.tile([1, m], f16, name="yya_s")
        xxb_s = singles.tile([128, nchunk], f32, name="xxb_s")
        ones_s = singles.tile([1, 128], f16, name="ones_s")
        ident = singles.tile([128, 128], f16, name="ident")
        cmh = singles.tile([128, m], f16, name="cmh")
        rm = singles.tile([128, nchunk], f32, name="rm")
        cs = singles.tile([128, m // 128], f32, name="cs")
        gsum = singles.tile([128, 1], f32, name="gsum")
        ones_col = singles.tile([128, 1], f32, name="ones_col")
        out_s = singles.tile([1, 1], f32, name="out_s")

        nc.vector.memset(ones_s, 1.0)
        nc.vector.memset(ones_col, 1.0)
        make_identity(nc, ident)
        nc.sync.dma_start(out=yya_s, in_=yya)
        nc.sync.dma_start(out=xxb_s, in_=xxb)

        # Column-grouped loads so matmuls can start before all data lands.
        gy = max(1, m // max(1, n_dma_groups))
        for g0 in range(0, m, gy):
            nc.sync.dma_start(out=ys0[:, g0:g0 + gy], in_=yt[0, :, g0:g0 + gy])
            nc.sync.dma_start(out=ys1[:, g0:g0 + gy], in_=yt[1, :, g0:g0 + gy])
        gx = max(1, n // max(1, n_dma_groups))
        for g0 in range(0, n, gx):
            nc.sync.dma_start(out=xs0[:, g0:g0 + gx], in_=xt[0, :, g0:g0 + gx])
            nc.sync.dma_start(out=xs1[:, g0:g0 + gx], in_=xt[1, :, g0:g0 + gx])

        for c in range(nchunk):
            ns = slice(c * 128, (c + 1) * 128)
            vh = vh_pool.tile([128, m], f16, name="vh")
            for j in range(nmb):
                ms = slice(j * MBLK, (j + 1) * MBLK)
                ps = ps_pool.tile([128, MBLK], mybir.dt.float32, name="ps")
                nc.tensor.matmul(ps, xs0[:, ns], ys0[:, ms], start=True, stop=False)
                nc.tensor.matmul(ps, xs1[:, ns], ys1[:, ms], start=False, stop=False)
                nc.tensor.matmul(ps, ones_s, yya_s[:, ms], start=False, stop=True)
                # V = Z - 0.5*yy - 0.5*xx  (xx via per-partition bias), fp16
                nc.scalar.activation(
                    out=vh[:, ms], in_=ps,
                    func=mybir.ActivationFunctionType.Identity,
                    bias=xxb_s[:, c:c + 1], scale=1.0,
                )
            # colmax running elementwise max across n-chunks
            if c == 0:
                nc.vector.tensor_copy(out=cmh, in_=vh)
            else:
                nc.vector.tensor_max(cmh, cmh, vh)
            # rowmax: fp16 2x-mode max tree, then one 1x reduce
            h = m // 2
            t1 = tree_pool.tile([128, h], f16, name="t1")
            nc.vector.tensor_max(t1, vh[:, :h], vh[:, h:])
            h //= 2
            t2 = tree_pool.tile([128, h], f16, name="t2")
            nc.vector.tensor_max(t2, t1[:, :h], t1[:, h:])
            h //= 2
            t3 = tree_pool.tile([128, h], f16, name="t3")
            nc.vector.tensor_max(t3, t2[:, :h], t2[:, h:])
            nc.vector.reduce_max(
                out=rm[:, c:c + 1], in_=t3, axis=mybir.AxisListType.X
            )

        # colmax: cross-partition max via PE transpose + free-dim reduce
        for k in range(m // 128):
            pt = tps_pool.tile([128, 128], f16, name="pt")
            nc.tensor.transpose(pt, cmh[:, k * 128:(k + 1) * 128], ident)
            nc.vector.reduce_max(
                out=cs[:, k:k + 1], in_=pt, axis=mybir.AxisListType.X
            )

        # grand total: sum everything, cross-partition sum via ones matmul
        gw = max(nchunk, m // 128)
        g_all = singles.tile([128, nchunk + m // 128], f32, name="g_all")
        nc.vector.tensor_copy(out=g_all[:, :nchunk], in_=rm)
        nc.vector.tensor_copy(out=g_all[:, nchunk:], in_=cs)
        nc.vector.reduce_sum(out=gsum, in_=g_all, axis=mybir.AxisListType.X)
        psf = fps_pool.tile([1, 1], mybir.dt.float32, name="psf")
        nc.tensor.matmul(psf, gsum, ones_col, start=True, stop=True)
        nc.scalar.mul(out_s, psf, -2.0)
        nc.sync.dma_start(out=out, in_=out_s)

    nc.compile()
    return nc


def prep_core_inputs(x, y):
    """Host-side layout prep for one batch element. x,y: [rows, 256] f32."""
    n, d = x.shape
    m, _ = y.shape
    xt = np.ascontiguousarray(x.T).reshape(2, 128, n)
    yt = np.ascontiguousarray(y.T).reshape(2, 128, m)
    xx = (x.astype(np.float32) * x).sum(axis=1, dtype=np.float32)
    yy = (y.astype(np.float32) * y).sum(axis=1, dtype=np.float32)
    xxb = np.ascontiguousarray((-0.5 * xx).reshape(n // 128, 128).T.astype(np.float32))
    yya = (-0.5 * yy).astype(np.float16).reshape(1, m)
    return {"xt": xt, "yt": yt, "xxb": xxb, "yya": yya}


def kernel(gts: np.ndarray, preds: np.ndarray) -> np.ndarray:
    gts = np.asarray(gts, dtype=np.float32)
    preds = np.asarray(preds, dtype=np.float32)
    assert gts.shape == (B, N, D) and preds.shape == (B, M, D)
    nc = build_chamfer()
    in_maps = [prep_core_inputs(gts[b], preds[b]) for b in range(B)]
    res = run_bass_kernel_spmd(nc, in_maps, core_ids=list(range(N_CORES)))
    total = np.float32(0.0)
    for b in range(B):
        total += np.float32(res.results[b]["out"][0, 0])
    return np.asarray(total, dtype=np.float32)


if __name__ == "__main__":
    # quick self-test on small shapes through CoreSim
    from concourse.bass_interp import CoreSim

    rng = np.random.default_rng(0)
    n_s, m_s = 256, 1024
    x = rng.standard_normal((n_s, D)).astype(np.float32)
    y = rng.standard_normal((m_s, D)).astype(np.float32)
    nc = build_chamfer(n_s, m_s, D, n_dma_groups=2)
    sim = CoreSim(nc)
    for k, v in prep_core_inputs(x, y).items():
        sim.tensor(k)[:] = v
    sim.simulate()
    got = float(sim.tensor("out")[0, 0])
    P = ((x[:, None, :] - y[None, :, :]) ** 2).sum(-1)
    want = P.min(0).sum() + P.min(1).sum()
    print(f"sim: {got:.4f}  ref: {want:.4f}  rel: {abs(got - want) / abs(want):.3e}")


# revision 7
# speedup vs baseline: 4518.2001x; 4518.2001x over previous
"""Chamfer loss Trainium2 Bass kernel.

Problem: gts [8, 4096, 256], preds [8, 4096, 256] (f32) ->
    loss = sum_b [ sum_m min_n P_b[n,m] + sum_n min_m P_b[n,m] ]
    where P_b[n,m] = ||gts[b,n] - preds[b,m]||^2.

Strategy (one batch element per NeuronCore, 8 cores):
  P[n,m] = xx[n] + yy[m] - 2*Z[n,m],  Z = x @ y^T.
  On device we compute V = Z - 0.5*yy[m] - 0.5*xx[n] = -P/2 directly:
    * Z via fp32r matmuls (x^T, y^T fed D-major; contraction over D in
      two K=128 chunks),
    * the -0.5*yy[m] term via one extra K=1 matmul row (ones (x) -0.5*yy),
    * the -0.5*xx[n] term via the ScalarE per-partition bias during the
      PSUM -> fp16 SBUF copy.
  Then min-reductions of P become max-reductions of V:
    rowmax[n] = max_m V  (free-dim max tree on VectorE)
    colmax[m] = max_n V  (elementwise running max across n-chunks, then a
                          PE transpose + free-dim reduce at the end)
  loss_b = -2 * (sum rowmax + sum colmax); partials summed on host.
"""

import numpy as np
from contextlib import ExitStack

import concourse.bass as bass
import concourse.mybir as mybir
import concourse.tile as tile
from concourse import bacc
from concourse.masks import make_identity
from concourse.bass_utils import run_bass_kernel_spmd

N_CORES = 8
B, N, M, D = 8, 4096, 4096, 256
MBLK = 512  # matmul moving-operand block (one PSUM bank of fp32)


def build_chamfer(n=N, m=M, d=D, n_dma_groups=4, repeat=1):
    """Build the per-core Bass module. x: n rows, y: m rows, dim d.

    repeat > 1 duplicates the whole compute body (same result written each
    time); used for differential wall-clock timing of the device portion.
    """
    assert d == 256, "kernel hardcodes two K=128 contraction chunks"
    assert n % 128 == 0 and m % MBLK == 0 and m % 128 == 0
    nchunk = n // 128
    nmb = m // MBLK
    f32, f32r, f16 = mybir.dt.float32, mybir.dt.float32r, mybir.dt.float16

    nc = bacc.Bacc("TRN2", target_bir_lowering=False, debug=False)
    xt = nc.dram_tensor("xt", [2, 128, n], f32r, kind="ExternalInput").ap()
    yt = nc.dram_tensor("yt", [2, 128, m], f32r, kind="ExternalInput").ap()
    xxb = nc.dram_tensor("xxb", [128, nchunk], f32, kind="ExternalInput").ap()
    yya = nc.dram_tensor("yya", [1, m], f16, kind="ExternalInput").ap()
    out = nc.dram_tensor("out", [1, 1], f32, kind="ExternalOutput").ap()

    with tile.TileContext(nc) as tc, ExitStack() as ctx:
        singles = ctx.enter_context(tc.tile_pool(name="singles", bufs=1))
        vh_pool = ctx.enter_context(tc.tile_pool(name="vh", bufs=2))
        tree_pool = ctx.enter_context(tc.tile_pool(name="tree", bufs=2))
        ps_pool = ctx.enter_context(tc.tile_pool(name="ps", bufs=4, space="PSUM"))
        tps_pool = ctx.enter_context(tc.tile_pool(name="tps", bufs=2, space="PSUM"))
        fps_pool = ctx.enter_context(tc.tile_pool(name="fps", bufs=1, space="PSUM"))

        # Static SBUF residents
        xs0 = singles.tile([128, n], f32r, name="xs0")
        xs1 = singles.tile([128, n], f32r, name="xs1")
        ys0 = singles.tile([128, m], f32r, name="ys0")
        ys1 = singles.tile([128, m], f32r, name="ys1")
        yya_s = singles.tile([1, m], f16, name="yya_s")
        xxb_s = singles.tile([128, nchunk], f32, name="xxb_s")
        ones_s = singles.tile([1, 128], f16, name="ones_s")
        ident = singles.tile([128, 128], f16, name="ident")
        cmh = singles.tile([128, m], f16, name="cmh")
        rm = singles.tile([128, nchunk], f32, name="rm")
        cs = singles.tile([128, m // 128], f32, name="cs")
        gsum = singles.tile([128, 1], f32, name="gsum")
        ones_col = singles.tile([128, 1], f32, name="ones_col")
        out_s = singles.tile([1, 1], f32, name="out_s")

        nc.vector.memset(ones_s, 1.0)
        nc.vector.memset(ones_col, 1.0)
        make_identity(nc, ident)
        nc.sync.dma_start(out=yya_s, in_=yya)
        nc.sync.dma_start(out=xxb_s, in_=xxb)

        # Column-grouped loads so matmuls can start before all data lands.
        gy = max(1, m // max(1, n_dma_groups))
        for g0 in range(0, m, gy):
            nc.sync.dma_start(out=ys0[:, g0:g0 + gy], in_=yt[0, :, g0:g0 + gy])
            nc.sync.dma_start(out=ys1[:, g0:g0 + gy], in_=yt[1, :, g0:g0 + gy])
        gx = max(1, n // max(1, n_dma_groups))
        for g0 in range(0, n, gx):
            nc.sync.dma_start(out=xs0[:, g0:g0 + gx], in_=xt[0, :, g0:g0 + gx])
            nc.sync.dma_start(out=xs1[:, g0:g0 + gx], in_=xt[1, :, g0:g0 + gx])

        for _rep in range(repeat):
          for c in range(nchunk):
            ns = slice(c * 128, (c + 1) * 128)
            vh = vh_pool.tile([128, m], f16, name="vh")
            for j in range(nmb):
                ms = slice(j * MBLK, (j + 1) * MBLK)
                ps = ps_pool.tile([128, MBLK], mybir.dt.float32, name="ps")
                nc.tensor.matmul(ps, xs0[:, ns], ys0[:, ms], start=True, stop=False)
                nc.tensor.matmul(ps, xs1[:, ns], ys1[:, ms], start=False, stop=False)
                nc.tensor.matmul(ps, ones_s, yya_s[:, ms], start=False, stop=True)
                # V = Z - 0.5*yy - 0.5*xx  (xx via per-partition bias), fp16
                nc.scalar.activation(
                    out=vh[:, ms], in_=ps,
                    func=mybir.ActivationFunctionType.Identity,
                    bias=xxb_s[:, c:c + 1], scale=1.0,
                )
            # colmax running elementwise max across n-chunks
            if c == 0:
                nc.vector.tensor_copy(out=cmh, in_=vh)
            else:
                nc.vector.tensor_max(cmh, cmh, vh)
            # rowmax: fp16 2x-mode max tree, then one 1x reduce
            h = m // 2
            t1 = tree_pool.tile([128, h], f16, name="t1")
            nc.vector.tensor_max(t1, vh[:, :h], vh[:, h:])
            h //= 2
            t2 = tree_pool.tile([128, h], f16, name="t2")
            nc.vector.tensor_max(t2, t1[:, :h], t1[:, h:])
            h //= 2
            t3 = tree_pool.tile([128, h], f16, name="t3")
            nc.vector.tensor_max(t3, t2[:, :h], t2[:, h:])
            nc.vector.reduce_max(
                out=rm[:, c:c + 1], in_=t3, axis=mybir.AxisListType.X
            )

          # colmax: cross-partition max via PE transpose + free-dim reduce
          for k in range(m // 128):
            pt = tps_pool.tile([128, 128], f16, name="pt")
            nc.tensor.transpose(pt, cmh[:, k * 128:(k + 1) * 128], ident)
            nc.vector.reduce_max(
                out=cs[:, k:k + 1], in_=pt, axis=mybir.AxisListType.X
            )

          # grand total: sum everything, cross-partition sum via ones matmul
          g_all = singles.tile([128, nchunk + m // 128], f32, name="g_all")
          nc.vector.tensor_copy(out=g_all[:, :nchunk], in_=rm)
          nc.vector.tensor_copy(out=g_all[:, nchunk:], in_=cs)
          nc.vector.reduce_sum(out=gsum, in_=g_all, axis=mybir.AxisListType.X)
          psf = fps_pool.tile([1, 1], mybir.dt.float32, name="psf")
          nc.tensor.matmul(psf, gsum, ones_col, start=True, stop=True)
          nc.scalar.mul(out_s, psf, -2.0)
          nc.sync.dma_start(out=out, in_=out_s)

    nc.compile()
    return nc


def prep_core_inputs(x, y):
    """Host-side layout prep for one batch element. x,y: [rows, 256] f32."""
    n, d = x.shape
    m, _ = y.shape
    xt = np.ascontiguousarray(x.T).reshape(2, 128, n)
    yt = np.ascontiguousarray(y.T).reshape(2, 128, m)
    xx = (x.astype(np.float32) * x).sum(axis=1, dtype=np.float32)
    yy = (y.astype(np.float32) * y).sum(axis=1, dtype=np.float32)
    xxb = np.ascontiguousarray((-0.5 * xx).reshape(n // 128, 128).T.astype(np.float32))
    yya = (-0.5 * yy).astype(np.float16).reshape(1, m)
    return {"xt": xt, "yt": yt, "xxb": xxb, "yya": yya}


def kernel(gts: np.ndarray, preds: np.ndarray) -> np.ndarray:
    gts = np.asarray(gts, dtype=np.float32)
    preds = np.asarray(preds, dtype=np.float32)
    assert gts.shape == (B, N, D) and preds.shape == (B, M, D)
    nc = build_chamfer()
    in_maps = [prep_core_inputs(gts[b], preds[b]) for b in range(B)]
    res = run_bass_kernel_spmd(nc, in_maps, core_ids=list(range(N_CORES)))
    total = np.float32(0.0)
    for b in range(B):
        total += np.float32(res.results[b]["out"][0, 0])
    return np.asarray(total, dtype=np.float32)


if __name__ == "__main__":
    # quick self-test on small shapes through CoreSim
    from concourse.bass_interp import CoreSim

    rng = np.random.default_rng(0)
    n_s, m_s = 256, 1024
    x = rng.standard_normal((n_s, D)).astype(np.float32)
    y = rng.standard_normal((m_s, D)).astype(np.float32)
    nc = build_chamfer(n_s, m_s, D, n_dma_groups=2)
    sim = CoreSim(nc)
    for k, v in prep_core_inputs(x, y).items():
        sim.tensor(k)[:] = v
    sim.simulate()
    got = float(sim.tensor("out")[0, 0])
    P = ((x[:, None, :] - y[None, :, :]) ** 2).sum(-1)
    want = P.min(0).sum() + P.min(1).sum()
    print(f"sim: {got:.4f}  ref: {want:.4f}  rel: {abs(got - want) / abs(want):.3e}")


# revision 57
# speedup vs baseline: 5586.9316x; 1.2365x over previous
"""Chamfer loss Trainium2 Bass kernel.

Problem: gts [8, 4096, 256], preds [8, 4096, 256] (f32) ->
    loss = sum_b [ sum_m min_n P_b[n,m] + sum_n min_m P_b[n,m] ]
    where P_b[n,m] = ||gts[b,n] - preds[b,m]||^2.

Strategy (one batch element per NeuronCore, 8 cores):
  P[n,m] = xx[n] + yy[m] - 2*Z[n,m],  Z = x @ y^T.
  On device we compute V = Z - 0.5*yy[m] - 0.5*xx[n] = -P/2 directly:
    * Z via fp32r matmuls (x^T, y^T fed D-major; contraction over D in
      two K=128 chunks),
    * the -0.5*yy[m] term via one extra K=1 matmul row (ones (x) -0.5*yy),
    * the -0.5*xx[n] term via the ScalarE per-partition bias during the
      PSUM -> fp16 SBUF copy.
  Then min-reductions of P become max-reductions of V:
    rowmax[n] = max_m V  (free-dim max tree on VectorE)
    colmax[m] = max_n V  (elementwise running max across n-chunks, then a
                          PE transpose + free-dim reduce at the end)
  loss_b = -2 * (sum rowmax + sum colmax); partials summed on host.
"""

import numpy as np
from contextlib import ExitStack

import concourse.bass as bass
import concourse.mybir as mybir
import concourse.tile as tile
from concourse import bacc
from concourse.masks import make_identity
from concourse.bass_utils import run_bass_kernel_spmd

N_CORES = 8
B, N, M, D = 8, 4096, 4096, 256
MBLK = 512  # matmul moving-operand block (one PSUM bank of fp32)


def build_chamfer(n=N, m=M, d=D, n_dma_groups=4, repeat=1, loop_n=0,
                  ps_bufs=8, vh_bufs=3, act_fd=MBLK, dve_aug_blocks=2):
    """Build the per-core Bass module. x: n rows, y: m rows, dim d.

    repeat > 1 duplicates the whole compute body (same result written each
    time); loop_n > 0 instead wraps the body in a device-side For_i loop.
    Both are used for differential wall-clock timing of the device portion.
    """
    assert d == 256, "kernel hardcodes two K=128 contraction chunks"
    assert n % 128 == 0 and m % MBLK == 0 and m % 128 == 0
    nchunk = n // 128
    nmb = m // MBLK
    f32, f32r, f16 = mybir.dt.float32, mybir.dt.float32r, mybir.dt.float16

    nc = bacc.Bacc("TRN2", target_bir_lowering=False, debug=False)
    dab = min(dve_aug_blocks, m // (2 * MBLK))
    pe_blocks = nmb - dab
    xt = nc.dram_tensor("xt", [2, 128, n], f32r, kind="ExternalInput").ap()
    yt = nc.dram_tensor("yt", [2, 128, m], f32r, kind="ExternalInput").ap()
    xxb = nc.dram_tensor("xxb", [128, nchunk], f32, kind="ExternalInput").ap()
    yya = nc.dram_tensor("yya", [128, m], f32r, kind="ExternalInput").ap()
    onesp = nc.dram_tensor("onesp", [128, 128], f32r, kind="ExternalInput").ap()
    if dab:
        yyh = nc.dram_tensor("yyh", [128, m], mybir.dt.float16,
                             kind="ExternalInput").ap()
    out = nc.dram_tensor("out", [1, 1], f32, kind="ExternalOutput").ap()

    with tile.TileContext(nc) as tc, ExitStack() as ctx:
        singles = ctx.enter_context(tc.tile_pool(name="singles", bufs=1))
        vh_pool = ctx.enter_context(tc.tile_pool(name="vh", bufs=vh_bufs))
        tree_pool = ctx.enter_context(tc.tile_pool(name="tree", bufs=2))
        ps_pool = ctx.enter_context(tc.tile_pool(name="ps", bufs=ps_bufs, space="PSUM"))

        # Static SBUF residents
        xs0 = singles.tile([128, n], f32r, name="xs0")
        xs1 = singles.tile([128, n], f32r, name="xs1")
        ys0 = singles.tile([128, m], f32r, name="ys0")
        ys1 = singles.tile([128, m], f32r, name="ys1")
        # padded K=128 aug operands: only row 0 is nonzero, so rows 1-127
        # of both sides contribute nothing to the accumulated product.
        yya_p = singles.tile([128, m], f32r, name="yya_p")
        xxb_s = singles.tile([128, nchunk], f32, name="xxb_s")
        ones_p = singles.tile([128, 128], f32r, name="ones_p")
        ident = singles.tile([128, 128], f16, name="ident")
        cmh = singles.tile([128, m], f16, name="cmh")
        rm = singles.tile([128, nchunk], f32, name="rm")
        cs = singles.tile([128, m // 128], f32, name="cs")
        gsum = singles.tile([128, 1], f32, name="gsum")
        ones_col = singles.tile([128, 1], f32, name="ones_col")
        out_s = singles.tile([1, 1], f32, name="out_s")

        nc.vector.memset(ones_col, 1.0)
        make_identity(nc, ident)
        nc.sync.dma_start(out=yya_p, in_=yya)
        nc.sync.dma_start(out=ones_p, in_=onesp)
        nc.sync.dma_start(out=xxb_s, in_=xxb)
        tail0 = pe_blocks * MBLK  # first column handled by the DVE yy path
        if dab:
            yyh_s = singles.tile([128, m - tail0], f16, name="yyh_s")
            nc.sync.dma_start(out=yyh_s, in_=yyh[:, tail0:])

        # Column-grouped loads so matmuls can start before all data lands.
        gy = max(1, m // max(1, n_dma_groups))
        for g0 in range(0, m, gy):
            nc.sync.dma_start(out=ys0[:, g0:g0 + gy], in_=yt[0, :, g0:g0 + gy])
            nc.sync.dma_start(out=ys1[:, g0:g0 + gy], in_=yt[1, :, g0:g0 + gy])
        gx = max(1, n // max(1, n_dma_groups))
        for g0 in range(0, n, gx):
            nc.sync.dma_start(out=xs0[:, g0:g0 + gx], in_=xt[0, :, g0:g0 + gx])
            nc.sync.dma_start(out=xs1[:, g0:g0 + gx], in_=xt[1, :, g0:g0 + gx])

        def body():
            for c in range(nchunk):
                ns = slice(c * 128, (c + 1) * 128)
                vh = vh_pool.tile([128, m], f16, name="vh")
                assert act_fd % MBLK == 0
                jg = act_fd // MBLK  # matmul tiles per ACT copy
                for j0 in range(0, nmb, jg):
                    ps = ps_pool.tile([128, act_fd], mybir.dt.float32, name="ps")
                    for j in range(j0, j0 + jg):
                        ms = slice(j * MBLK, (j + 1) * MBLK)
                        pslice = ps[:, (j - j0) * MBLK:(j - j0 + 1) * MBLK]
                        nc.tensor.matmul(pslice, xs0[:, ns], ys0[:, ms],
                                         start=True, stop=False)
                        nc.tensor.matmul(pslice, xs1[:, ns], ys1[:, ms],
                                         start=False, stop=j >= pe_blocks)
                        if j < pe_blocks:
                            nc.tensor.matmul(pslice, ones_p, yya_p[:, ms],
                                             start=False, stop=True)
                    # V = Z - 0.5*yy - 0.5*xx  (xx via per-partition bias), fp16
                    # (tail blocks get their -0.5*yy on the DVE below)
                    nc.scalar.activation(
                        out=vh[:, j0 * MBLK:j0 * MBLK + act_fd], in_=ps,
                        func=mybir.ActivationFunctionType.Identity,
                        bias=xxb_s[:, c:c + 1], scale=1.0,
                    )
                if dab:
                    wt = tree_pool.tile([128, m - tail0], f16, name="wt")
                    nc.vector.tensor_add(wt, vh[:, tail0:], yyh_s)
                # colmax running elementwise max across n-chunks
                if c == 0:
                    nc.vector.tensor_copy(out=cmh[:, :tail0], in_=vh[:, :tail0])
                    if dab:
                        nc.vector.tensor_copy(out=cmh[:, tail0:], in_=wt)
                else:
                    nc.vector.tensor_max(cmh[:, :tail0], cmh[:, :tail0],
                                         vh[:, :tail0])
                    if dab:
                        nc.vector.tensor_max(cmh[:, tail0:], cmh[:, tail0:], wt)
                # rowmax: fp16 2x-mode max tree, then one 1x reduce
                h = m // 2
                t1 = tree_pool.tile([128, h], f16, name="t1")
                cut = tail0 - h  # columns of the second half still in vh
                if cut > 0:
                    nc.vector.tensor_max(t1[:, :cut], vh[:, :cut], vh[:, h:tail0])
                if dab:
                    nc.vector.tensor_max(t1[:, cut:], vh[:, cut:h], wt)
                h //= 2
                t2 = tree_pool.tile([128, h], f16, name="t2")
                nc.vector.tensor_max(t2, t1[:, :h], t1[:, h:])
                h //= 2
                t3 = tree_pool.tile([128, h], f16, name="t3")
                nc.vector.tensor_max(t3, t2[:, :h], t2[:, h:])
                h //= 2
                t4 = tree_pool.tile([128, h], f16, name="t4")
                nc.vector.tensor_max(t4, t3[:, :h], t3[:, h:])
                nc.vector.reduce_max(
                    out=rm[:, c:c + 1], in_=t4, axis=mybir.AxisListType.X
                )

            # colmax: cross-partition max via PE transpose + free-dim reduce;
            # 4 transposes packed per PSUM slot, one packed reduce each
            # (transpose/final tiles share the main PSUM slots via tag)
            kq = 4
            for k0 in range(0, m // 128, kq):
                pt = ps_pool.tile([128, kq, 128], f16, name="pt", tag="ps")
                for q in range(kq):
                    nc.tensor.transpose(
                        pt[:, q, :], cmh[:, (k0 + q) * 128:(k0 + q + 1) * 128],
                        ident)
                nc.vector.reduce_max(
                    out=cs[:, k0:k0 + kq], in_=pt, axis=mybir.AxisListType.X
                )

            # grand total: sum everything, cross-partition sum via ones matmul
            g_all = tree_pool.tile([128, nchunk + m // 128], f32, name="g_all")
            nc.vector.tensor_copy(out=g_all[:, :nchunk], in_=rm)
            nc.vector.tensor_copy(out=g_all[:, nchunk:], in_=cs)
            nc.vector.reduce_sum(out=gsum, in_=g_all, axis=mybir.AxisListType.X)
            psf = ps_pool.tile([1, 1], mybir.dt.float32, name="psf", tag="ps")
            nc.tensor.matmul(psf, gsum, ones_col, start=True, stop=True)
            nc.scalar.mul(out_s, psf, -2.0)
            nc.sync.dma_start(out=out, in_=out_s)

        if loop_n:
            with tc.For_i(0, loop_n, 1) as _i:
                body()
        else:
            for _rep in range(repeat):
                body()

    nc.compile()
    return nc


def build_micro(n=N, m=M, d=D, loop_n=8, mode="mm2"):
    """Engine-isolation microbenches, all sharing the chamfer data layout.

    mode: mm2 (2 fp32r MM/tile), mm3 (+fp16 aug MM), mmbf (2 bf16 MM/tile),
          act (PSUM->fp16 SBUF copies w/ bias), dve (max tree + colmax),
          actdve (act + dve chained as in the real kernel)
    """
    nchunk, nmb = n // 128, m // MBLK
    f32, f32r, f16 = mybir.dt.float32, mybir.dt.float32r, mybir.dt.float16
    bf16 = mybir.dt.bfloat16
    augdt = f32r if mode.endswith("r") else f16
    nc = bacc.Bacc("TRN2", target_bir_lowering=False, debug=False)
    xt = nc.dram_tensor("xt", [2, 128, n], f32r, kind="ExternalInput").ap()
    yt = nc.dram_tensor("yt", [2, 128, m], f32r, kind="ExternalInput").ap()
    xxb = nc.dram_tensor("xxb", [128, nchunk], f32, kind="ExternalInput").ap()
    yya = nc.dram_tensor("yya", [1, m], augdt, kind="ExternalInput").ap()
    out = nc.dram_tensor("out", [1, 1], f32, kind="ExternalOutput").ap()

    with tile.TileContext(nc) as tc, ExitStack() as ctx:
        singles = ctx.enter_context(tc.tile_pool(name="singles", bufs=1))
        vh_pool = ctx.enter_context(tc.tile_pool(name="vh", bufs=2))
        tree_pool = ctx.enter_context(tc.tile_pool(name="tree", bufs=2))
        ps_pool = ctx.enter_context(tc.tile_pool(name="ps", bufs=6, space="PSUM"))
        xs0 = singles.tile([128, n], f32r, name="xs0")
        xs1 = singles.tile([128, n], f32r, name="xs1")
        ys0 = singles.tile([128, m], f32r, name="ys0")
        ys1 = singles.tile([128, m], f32r, name="ys1")
        yya_s = singles.tile([1, m], augdt, name="yya_s")
        xxb_s = singles.tile([128, nchunk], f32, name="xxb_s")
        if augdt == f32r:
            ones_f32 = singles.tile([1, 128], f32, name="ones_f32")
            nc.vector.memset(ones_f32, 1.0)
            ones_s = ones_f32.bitcast(f32r)
        else:
            ones_s = singles.tile([1, 128], augdt, name="ones_s")
            nc.vector.memset(ones_s, 1.0)
        cmh = singles.tile([128, m], f16, name="cmh")
        rm = singles.tile([128, nchunk], f32, name="rm")
        sink = singles.tile([128, nchunk * nmb], f32, name="sink")
        out_s = singles.tile([1, 1], f32, name="out_s")
        ones_pad = singles.tile([128, 128], f32, name="ones_pad")
        nc.vector.memset(ones_pad, 0.0)
        nc.vector.memset(ones_pad[0:1, :], 1.0)
        nc.vector.memset(cmh, 0.0)
        nc.sync.dma_start(out=yya_s, in_=yya)
        nc.sync.dma_start(out=xxb_s, in_=xxb)
        for cd in range(2):
            nc.sync.dma_start(out=(xs0, xs1)[cd], in_=xt[cd])
            nc.sync.dma_start(out=(ys0, ys1)[cd], in_=yt[cd])
        xs0b = xs0.bitcast(bf16) if mode == "mmbf" else None

        def body():
            for c in range(nchunk):
                ns = slice(c * 128, (c + 1) * 128)
                vh = vh_pool.tile([128, m], f16, name="vh")
                if mode == "mm3q":
                    # all 8 banks of main MMs, then 2 quads of row-group
                    # packed K=32 aug MMs (garbage values; timing only)
                    pss = []
                    for j in range(nmb):
                        ms = slice(j * MBLK, (j + 1) * MBLK)
                        ps = ps_pool.tile([128, MBLK], mybir.dt.float32,
                                          name="ps", bufs=8)
                        nc.tensor.matmul(ps, xs0[:, ns], ys0[:, ms],
                                         start=True, stop=False)
                        nc.tensor.matmul(ps, xs1[:, ns], ys1[:, ms],
                                         start=False, stop=False)
                        pss.append(ps)
                    for j in range(nmb):
                        ms = slice(j * MBLK, (j + 1) * MBLK)
                        q = j % 4
                        nc.tensor.matmul(
                            pss[j], xs0[32 * q:32 * (q + 1), ns],
                            ys0[32 * q:32 * (q + 1), ms],
                            start=False, stop=True,
                            tile_position=(32 * q, 0))
                    for j in range(nmb):
                        nc.scalar.activation(
                            out=sink[:, c * nmb + j:c * nmb + j + 1],
                            in_=pss[j][:, 0:1],
                            func=mybir.ActivationFunctionType.Copy,
                        )
                if mode in ("mm2", "mm3", "mm3r", "mm3p", "mm3b", "mmbf", "mmact"):
                    for j in range(nmb):
                        ms = slice(j * MBLK, (j + 1) * MBLK)
                        ps = ps_pool.tile([128, MBLK], mybir.dt.float32, name="ps")
                        if mode == "mmact":
                            nc.tensor.matmul(ps, xs0[:, ns], ys0[:, ms],
                                             start=True, stop=False)
                            nc.tensor.matmul(ps, xs1[:, ns], ys1[:, ms],
                                             start=False, stop=False)
                            nc.tensor.matmul(ps, ones_s, yya_s[:, ms],
                                             start=False, stop=True)
                            nc.scalar.activation(
                                out=vh[:, ms], in_=ps,
                                func=mybir.ActivationFunctionType.Identity,
                                bias=xxb_s[:, c:c + 1], scale=1.0,
                            )
                            continue
                        if mode == "mmbf":
                            # values are garbage (bitcast) — timing only
                            nc.tensor.matmul(
                                ps, xs0.bitcast(bf16)[:, ns],
                                ys0.bitcast(bf16)[:, ms],
                                start=True, stop=False)
                            nc.tensor.matmul(
                                ps, xs1.bitcast(bf16)[:, ns],
                                ys1.bitcast(bf16)[:, ms],
                                start=False, stop=True)
                        else:
                            nc.tensor.matmul(ps, xs0[:, ns], ys0[:, ms],
                                             start=True, stop=False)
                            nc.tensor.matmul(ps, xs1[:, ns], ys1[:, ms],
                                             start=False, stop=mode == "mm2")
                            if mode in ("mm3", "mm3r"):
                                nc.tensor.matmul(ps, ones_s, yya_s[:, ms],
                                                 start=False, stop=True)
                            elif mode == "mm3p":
                                # padded K=128 aug (timing proxy: ys0 as rhs)
                                nc.tensor.matmul(ps, ones_pad.bitcast(f32r),
                                                 ys0[:, ms],
                                                 start=False, stop=True)
                            elif mode == "mm3b":
                                # padded K=128 fp16 aug (ldweights+MM pair)
                                nc.tensor.matmul(ps, ones_pad.bitcast(f16)[:, :128],
                                                 ys0.bitcast(f16)[:, ms],
                                                 start=False, stop=True)
                        nc.scalar.activation(
                            out=sink[:, c * nmb + j:c * nmb + j + 1],
                            in_=ps[:, 0:1],
                            func=mybir.ActivationFunctionType.Copy,
                        )
                if mode in ("act", "actdve"):
                    for j in range(nmb):
                        ms = slice(j * MBLK, (j + 1) * MBLK)
                        nc.scalar.activation(
                            out=vh[:, ms], in_=cmh[:, ms],
                            func=mybir.ActivationFunctionType.Identity,
                            bias=xxb_s[:, c:c + 1], scale=1.0,
                        )
                if mode in ("dve", "actdve"):
                    src = vh if mode == "actdve" else cmh
                    h = m // 2
                    t1 = tree_pool.tile([128, h], f16, name="t1")
                    nc.vector.tensor_max(t1, src[:, :h], src[:, h:])
                    h //= 2
                    t2 = tree_pool.tile([128, h], f16, name="t2")
                    nc.vector.tensor_max(t2, t1[:, :h], t1[:, h:])
                    h //= 2
                    t3 = tree_pool.tile([128, h], f16, name="t3")
                    nc.vector.tensor_max(t3, t2[:, :h], t2[:, h:])
                    nc.vector.reduce_max(out=rm[:, c:c + 1], in_=t3,
                                         axis=mybir.AxisListType.X)
                    if mode == "dve":
                        nc.vector.tensor_max(cmh, cmh, cmh)
                    else:
                        nc.vector.tensor_max(cmh, cmh, vh)

        with tc.For_i(0, loop_n, 1) as _i:
            body()
        final_src = {"mm2": sink, "mm3": sink, "mm3r": sink, "mm3p": sink,
                     "mm3q": sink, "mm3b": sink, "mmbf": sink, "act": cmh,
                     "dve": rm, "actdve": rm, "mmact": xxb_s}[mode]
        nc.scalar.copy(out_s, final_src[0:1, 0:1])
        nc.sync.dma_start(out=out, in_=out_s)
    nc.compile()
    return nc


def prep_core_inputs(x, y):
    """Host-side layout prep for one batch element. x,y: [rows, 256] f32."""
    n, d = x.shape
    m, _ = y.shape
    xt = np.ascontiguousarray(x.T).reshape(2, 128, n)
    yt = np.ascontiguousarray(y.T).reshape(2, 128, m)
    xx = (x.astype(np.float32) * x).sum(axis=1, dtype=np.float32)
    yy = (y.astype(np.float32) * y).sum(axis=1, dtype=np.float32)
    xxb = np.ascontiguousarray((-0.5 * xx).reshape(n // 128, 128).T.astype(np.float32))
    yya = np.zeros((128, m), dtype=np.float32)
    yya[0, :] = -0.5 * yy
    onesp = np.zeros((128, 128), dtype=np.float32)
    onesp[0, :] = 1.0
    yyh = np.ascontiguousarray(
        np.broadcast_to((-0.5 * yy).astype(np.float16), (128, m)))
    return {"xt": xt, "yt": yt, "xxb": xxb, "yya": yya, "onesp": onesp,
            "yyh": yyh}


def kernel(gts: np.ndarray, preds: np.ndarray) -> np.ndarray:
    gts = np.asarray(gts, dtype=np.float32)
    preds = np.asarray(preds, dtype=np.float32)
    assert gts.shape == (B, N, D) and preds.shape == (B, M, D)
    nc = build_chamfer()
    in_maps = [prep_core_inputs(gts[b], preds[b]) for b in range(B)]
    res = run_bass_kernel_spmd(nc, in_maps, core_ids=list(range(N_CORES)))
    total = np.float32(0.0)
    for b in range(B):
        total += np.float32(res.results[b]["out"][0, 0])
    return np.asarray(total, dtype=np.float32)


if __name__ == "__main__":
    # quick self-test on small shapes through CoreSim
    from concourse.bass_interp import CoreSim

    rng = np.random.default_rng(0)
    n_s, m_s = 256, 1024
    x = rng.standard_normal((n_s, D)).astype(np.float32)
    y = rng.standard_normal((m_s, D)).astype(np.float32)
    nc = build_chamfer(n_s, m_s, D, n_dma_groups=2)
    sim = CoreSim(nc)
    for k, v in prep_core_inputs(x, y).items():
        sim.tensor(k)[:] = v
    sim.simulate()
    got = float(sim.tensor("out")[0, 0])
    P = ((x[:, None, :] - y[None, :, :]) ** 2).sum(-1)
    want = P.min(0).sum() + P.min(1).sum()
    print(f"sim: {got:.4f}  ref: {want:.4f}  rel: {abs(got - want) / abs(want):.3e}")
